# revision 16
# baseline (speedup 1.0000x reference)
"""Trainium2 Bass kernel for nn_DCT: YCbCr 3x3 channel mix + 8x8 block DCT
(stride 8) + repeated min/max normalization collapsed to a per-channel affine.

Sharding: pure data parallel, batch 32 -> 4 samples on each of 8 NeuronCores.

v3 dataflow — direct 2D DCT, everything inside one matmul stage:
  - Host shuffles x to block-pixel layout: xa[s, (ci01,i,j)=128, blk=4096],
    xb[s, (ci2,i,j)+ones=65, blk=4096] (row 64 = 1.0; blk = hb*64+wb).
  - Per-sample rhs constants carry mix, both DCTs, the affine scale AND bias:
      K[(ci,i,j),(co,u,v)] = y[co,ci]*D[u,i]*D[v,j]*s_aff[smp,co,u,v]
      K1 = K rows 0:128, K2 = K rows 128:192 + row 64 = b_aff[smp].
  - Per (s, group of 128 blocks): two accumulating matmuls
      ps[blk128, (co,u,v)] = xa_g^T @ K1 + xb_g^T @ K2   (f32 PSUM)
    give the FINAL normalized DCT output directly.
  - PSUM -> SBUF bf16 copies split between Scalar and Vector engines.
  - One contiguous 1.5MB DMA out per sample; host untangles.
"""

import math
import sys

import numpy as np

for _p in ("/opt/trn_rl_repo", "/opt/pypackages"):
    if _p not in sys.path:
        sys.path.insert(0, _p)

N = 8
IN_CH = 3
EPS = 1e-6
B_FULL = 32
H = 512
W = 512
NCORES = 8
BPC = B_FULL // NCORES  # samples per core
NBLK = 4096  # 64x64 blocks per image
NGRP = NBLK // 128  # 32 groups of 128 blocks

_CACHED_NC = None


def _dct_basis(n=N):
    u = np.arange(n)
    i = np.arange(n)
    b = np.cos(np.pi * np.outer(u, i + 0.5) / n)
    c = np.full(n, math.sqrt(2.0 / n))
    c[0] = math.sqrt(1.0 / n)
    return b * c[:, None]


def _affine_coeffs(max_, min_):
    """Closed form of t -> (t - min)/d applied B_FULL times: out = s*dct + b."""
    m = np.asarray(max_, np.float32)[..., 0, 0]
    n = np.asarray(min_, np.float32)[..., 0, 0]
    d = (m - n + np.float32(EPS)).astype(np.float64)
    r = 1.0 / d
    s = r**B_FULL
    b = -n.astype(np.float64) * (r * (1.0 - s) / (1.0 - r))
    return s, b  # [B, 192] f64


def _build_nc():
    import concourse.mybir as mybir
    import concourse.tile as tile
    from concourse import bacc
    from contextlib import ExitStack

    f32 = mybir.dt.float32
    bf16 = mybir.dt.bfloat16
    nc = bacc.Bacc()
    xa_t = nc.declare_dram_parameter("xa", [BPC, 128, NBLK], bf16, isOutput=False)
    xb_t = nc.declare_dram_parameter("xb", [BPC, 65, NBLK], bf16, isOutput=False)
    k1_t = nc.declare_dram_parameter("k1", [BPC, 128, 192], bf16, isOutput=False)
    k2_t = nc.declare_dram_parameter("k2", [BPC, 65, 192], bf16, isOutput=False)
    # out[s, p=blk%128, g=blk//128, (co,u,v)]
    out_t = nc.declare_dram_parameter("out", [BPC, 128, NGRP, 192], bf16, isOutput=True)

    with ExitStack() as ctx:
        tc = ctx.enter_context(tile.TileContext(nc))
        consts = ctx.enter_context(tc.tile_pool(name="consts", bufs=1))
        xap = ctx.enter_context(tc.tile_pool(name="xap", bufs=8))
        xbp = ctx.enter_context(tc.tile_pool(name="xbp", bufs=4))
        psp = ctx.enter_context(tc.tile_pool(name="psp", bufs=3, space="PSUM"))
        outp = ctx.enter_context(tc.tile_pool(name="outp", bufs=6))

        k1_sb = consts.tile([128, BPC, 192], bf16)
        nc.gpsimd.dma_start(out=k1_sb, in_=k1_t[:].rearrange("s k n -> k s n"))
        k2_sb = consts.tile([65, BPC, 192], bf16)
        nc.gpsimd.dma_start(out=k2_sb, in_=k2_t[:].rearrange("s k n -> k s n"))

        for s in range(BPC):
            # per-chunk tiles: first matmuls only wait on a 256KB load, and
            # prefetch depth is gated at chunk granularity
            xa_q = []
            for q in range(4):
                t = xap.tile([128, 1024], bf16)
                nc.sync.dma_start(out=t, in_=xa_t[s][:, q * 1024 : (q + 1) * 1024])
                xa_q.append(t)
            xb_q = []
            for q in range(2):
                t = xbp.tile([65, 2048], bf16)
                nc.gpsimd.dma_start(out=t, in_=xb_t[s][:, q * 2048 : (q + 1) * 2048])
                xb_q.append(t)
            for t4 in range(NGRP // 4):  # 8 PSUM tiles x 4 groups
                ps = psp.tile([128, 4, 256], f32)  # 2 banks; use [:, q, 0:192]
                for q in range(4):
                    g = t4 * 4 + q
                    ga, go = divmod(g, 8)
                    gb, gbo = divmod(g, 16)
                    nc.tensor.matmul(
                        ps[:, q, 0:192],
                        lhsT=xa_q[ga][:, go * 128 : (go + 1) * 128],
                        rhs=k1_sb[:, s],
                        start=True,
                        stop=False,
                    )
                    nc.tensor.matmul(
                        ps[:, q, 0:192],
                        lhsT=xb_q[gb][:, gbo * 128 : (gbo + 1) * 128],
                        rhs=k2_sb[:, s],
                        start=False,
                        stop=True,
                    )
                dst = outp.tile([128, 4, 192], bf16)
                if t4 % 2 == 0:
                    nc.scalar.copy(out=dst, in_=ps[:, :, 0:192])
                else:
                    nc.vector.tensor_copy(out=dst, in_=ps[:, :, 0:192])
                # out-DMA per copy, issued by the same engine that produced it:
                # keeps the sync queue free for input prefetch (no head-of-line
                # blocking of the next sample's loads behind this sample's out)
                if t4 % 2 == 0:
                    nc.scalar.dma_start(
                        out=out_t[s][:, t4 * 4 : (t4 + 1) * 4], in_=dst
                    )
                else:
                    nc.gpsimd.dma_start(
                        out=out_t[s][:, t4 * 4 : (t4 + 1) * 4], in_=dst
                    )
    return nc


def _get_nc():
    global _CACHED_NC
    if _CACHED_NC is None:
        nc = _build_nc()
        if not nc.is_finalized():
            nc.finalize()
        _CACHED_NC = nc
    return _CACHED_NC


def _make_in_maps(x, max_, min_, ycbcr_w):
    import ml_dtypes

    bf16 = ml_dtypes.bfloat16
    x16 = np.asarray(x, np.float32).astype(bf16)
    # block-pixel layout: [B, (ci,i,j)=192, blk=(hb,wb)=4096]
    xd = x16.reshape(-1, 3, 64, 8, 64, 8)  # s, ci, hb, i, wb, j
    xd = np.ascontiguousarray(xd.transpose(0, 1, 3, 5, 2, 4))  # s, ci, i, j, hb, wb
    xd = xd.reshape(-1, 192, NBLK)
    ones = np.ones((xd.shape[0], 1, NBLK), bf16)
    xa = np.ascontiguousarray(xd[:, 0:128])
    xb = np.ascontiguousarray(np.concatenate([xd[:, 128:192], ones], axis=1))

    s_aff, b_aff = _affine_coeffs(max_, min_)  # [B, 192] f64 (co,u,v)
    D = _dct_basis()  # [u, i] f64
    y = np.asarray(ycbcr_w, np.float64)  # [co, ci]
    # K[(ci,i,j), (co,u,v)] * s_aff[smp, (co,u,v)]
    kbase = np.einsum("oc,ui,vj->cijouv", y, D, D).reshape(192, 192)
    ks = kbase[None, :, :] * s_aff[:, None, :]  # [B, 192, 192]
    k1 = ks[:, 0:128]
    k2 = np.concatenate([ks[:, 128:192], b_aff[:, None, :]], axis=1)  # [B, 65, 192]

    in_maps = []
    for core in range(NCORES):
        sl = slice(core * BPC, (core + 1) * BPC)
        in_maps.append(
            {
                "xa": xa[sl],
                "xb": xb[sl],
                "k1": k1[sl].astype(bf16),
                "k2": k2[sl].astype(bf16),
            }
        )
    return in_maps


def kernel(x, max_, min_, ycbcr_w, dct_w):
    from concourse.bass_utils import run_bass_kernel_spmd

    nc = _get_nc()
    in_maps = _make_in_maps(x, max_, min_, ycbcr_w)
    res = run_bass_kernel_spmd(nc, in_maps, core_ids=list(range(NCORES)))
    out = np.concatenate([res.results[i]["out"] for i in range(NCORES)], axis=0)
    return _untangle(out)


def _untangle(dev_out):
    """[B, p=128, g=32, 192] device layout -> [B, 192, 64, 64] f32."""
    v = np.asarray(dev_out).astype(np.float32)
    v = v.transpose(0, 2, 1, 3)  # s, g, p, (co,u,v) ; blk = g*128+p = hb*64+wb
    v = v.reshape(-1, 64, 64, 3, 8, 8)  # s, hb, wb, co, u, v
    v = v.transpose(0, 3, 4, 5, 1, 2)  # s, co, u, v, hb, wb
    return np.ascontiguousarray(v.reshape(-1, 192, 64, 64))


# revision 19
# speedup vs baseline: 1.0758x; 1.0758x over previous
"""Trainium2 Bass kernel for nn_DCT: YCbCr 3x3 channel mix + 8x8 block DCT
(stride 8) + repeated min/max normalization collapsed to a per-channel affine.

Sharding: pure data parallel, batch 32 -> 4 samples on each of 8 NeuronCores.

v3 dataflow — direct 2D DCT, everything inside one matmul stage:
  - Host shuffles x to block-pixel layout: xa[s, (ci01,i,j)=128, blk=4096],
    xb[s, (ci2,i,j)+ones=65, blk=4096] (row 64 = 1.0; blk = hb*64+wb).
  - Per-sample rhs constants carry mix, both DCTs, the affine scale AND bias:
      K[(ci,i,j),(co,u,v)] = y[co,ci]*D[u,i]*D[v,j]*s_aff[smp,co,u,v]
      K1 = K rows 0:128, K2 = K rows 128:192 + row 64 = b_aff[smp].
  - Per (s, group of 128 blocks): two accumulating matmuls
      ps[blk128, (co,u,v)] = xa_g^T @ K1 + xb_g^T @ K2   (f32 PSUM)
    give the FINAL normalized DCT output directly.
  - PSUM -> SBUF bf16 copies split between Scalar and Vector engines.
  - One contiguous 1.5MB DMA out per sample; host untangles.
"""

import math
import sys

import numpy as np

for _p in ("/opt/trn_rl_repo", "/opt/pypackages"):
    if _p not in sys.path:
        sys.path.insert(0, _p)

N = 8
IN_CH = 3
EPS = 1e-6
B_FULL = 32
H = 512
W = 512
NCORES = 8
BPC = B_FULL // NCORES  # samples per core
NBLK = 4096  # 64x64 blocks per image
NGRP = NBLK // 128  # 32 groups of 128 blocks

_CACHED_NC = None


def _dct_basis(n=N):
    u = np.arange(n)
    i = np.arange(n)
    b = np.cos(np.pi * np.outer(u, i + 0.5) / n)
    c = np.full(n, math.sqrt(2.0 / n))
    c[0] = math.sqrt(1.0 / n)
    return b * c[:, None]


def _affine_coeffs(max_, min_):
    """Closed form of t -> (t - min)/d applied B_FULL times: out = s*dct + b."""
    m = np.asarray(max_, np.float32)[..., 0, 0]
    n = np.asarray(min_, np.float32)[..., 0, 0]
    d = (m - n + np.float32(EPS)).astype(np.float64)
    r = 1.0 / d
    s = r**B_FULL
    b = -n.astype(np.float64) * (r * (1.0 - s) / (1.0 - r))
    return s, b  # [B, 192] f64


def _build_nc():
    import concourse.mybir as mybir
    import concourse.tile as tile
    from concourse import bacc
    from contextlib import ExitStack

    f32 = mybir.dt.float32
    bf16 = mybir.dt.bfloat16
    nc = bacc.Bacc()
    xa_t = nc.declare_dram_parameter("xa", [BPC, 128, NBLK], bf16, isOutput=False)
    xb_t = nc.declare_dram_parameter("xb", [BPC, 65, NBLK], bf16, isOutput=False)
    k1_t = nc.declare_dram_parameter("k1", [BPC, 128, 192], bf16, isOutput=False)
    k2_t = nc.declare_dram_parameter("k2", [BPC, 65, 192], bf16, isOutput=False)
    # out[s, p=blk%128, g=blk//128, (co,u,v)]
    out_t = nc.declare_dram_parameter("out", [BPC, 128, NGRP, 192], bf16, isOutput=True)

    with ExitStack() as ctx:
        tc = ctx.enter_context(tile.TileContext(nc))
        consts = ctx.enter_context(tc.tile_pool(name="consts", bufs=1))
        xap = ctx.enter_context(tc.tile_pool(name="xap", bufs=2))
        xbp = ctx.enter_context(tc.tile_pool(name="xbp", bufs=2))
        psp = ctx.enter_context(tc.tile_pool(name="psp", bufs=3, space="PSUM"))
        outp = ctx.enter_context(tc.tile_pool(name="outp", bufs=6))

        # queue discipline: sync issues ONLY input loads, gpsimd ONLY output
        # stores, scalar/vector only their copies — no head-of-line blocking
        # of the next sample's loads behind compute-dependent stores.
        k1_sb = consts.tile([128, BPC, 192], bf16)
        nc.sync.dma_start(out=k1_sb, in_=k1_t[:].rearrange("s k n -> k s n"))
        k2_sb = consts.tile([65, BPC, 192], bf16)
        nc.sync.dma_start(out=k2_sb, in_=k2_t[:].rearrange("s k n -> k s n"))

        for s in range(BPC):
            xa = xap.tile([128, NBLK], bf16)
            for q in range(4):
                nc.sync.dma_start(
                    out=xa[:, q * 1024 : (q + 1) * 1024],
                    in_=xa_t[s][:, q * 1024 : (q + 1) * 1024],
                )
            xb = xbp.tile([65, NBLK], bf16)
            for q in range(2):
                nc.sync.dma_start(
                    out=xb[:, q * 2048 : (q + 1) * 2048],
                    in_=xb_t[s][:, q * 2048 : (q + 1) * 2048],
                )
            for t4 in range(NGRP // 4):  # 8 PSUM tiles x 4 groups
                ps = psp.tile([128, 4, 256], f32)  # 2 banks; use [:, q, 0:192]
                for q in range(4):
                    g = t4 * 4 + q
                    nc.tensor.matmul(
                        ps[:, q, 0:192],
                        lhsT=xa[:, g * 128 : (g + 1) * 128],
                        rhs=k1_sb[:, s],
                        start=True,
                        stop=False,
                    )
                    nc.tensor.matmul(
                        ps[:, q, 0:192],
                        lhsT=xb[:, g * 128 : (g + 1) * 128],
                        rhs=k2_sb[:, s],
                        start=False,
                        stop=True,
                    )
                dst = outp.tile([128, 4, 192], bf16)
                if t4 % 2 == 0:
                    nc.scalar.copy(out=dst, in_=ps[:, :, 0:192])
                else:
                    nc.vector.tensor_copy(out=dst, in_=ps[:, :, 0:192])
                nc.gpsimd.dma_start(
                    out=out_t[s][:, t4 * 4 : (t4 + 1) * 4], in_=dst
                )
    return nc


def _get_nc():
    global _CACHED_NC
    if _CACHED_NC is None:
        nc = _build_nc()
        if not nc.is_finalized():
            nc.finalize()
        _CACHED_NC = nc
    return _CACHED_NC


def _make_in_maps(x, max_, min_, ycbcr_w):
    import ml_dtypes

    bf16 = ml_dtypes.bfloat16
    x16 = np.asarray(x, np.float32).astype(bf16)
    # block-pixel layout: [B, (ci,i,j)=192, blk=(hb,wb)=4096]
    xd = x16.reshape(-1, 3, 64, 8, 64, 8)  # s, ci, hb, i, wb, j
    xd = np.ascontiguousarray(xd.transpose(0, 1, 3, 5, 2, 4))  # s, ci, i, j, hb, wb
    xd = xd.reshape(-1, 192, NBLK)
    ones = np.ones((xd.shape[0], 1, NBLK), bf16)
    xa = np.ascontiguousarray(xd[:, 0:128])
    xb = np.ascontiguousarray(np.concatenate([xd[:, 128:192], ones], axis=1))

    s_aff, b_aff = _affine_coeffs(max_, min_)  # [B, 192] f64 (co,u,v)
    D = _dct_basis()  # [u, i] f64
    y = np.asarray(ycbcr_w, np.float64)  # [co, ci]
    # K[(ci,i,j), (co,u,v)] * s_aff[smp, (co,u,v)]
    kbase = np.einsum("oc,ui,vj->cijouv", y, D, D).reshape(192, 192)
    ks = kbase[None, :, :] * s_aff[:, None, :]  # [B, 192, 192]
    k1 = ks[:, 0:128]
    k2 = np.concatenate([ks[:, 128:192], b_aff[:, None, :]], axis=1)  # [B, 65, 192]

    in_maps = []
    for core in range(NCORES):
        sl = slice(core * BPC, (core + 1) * BPC)
        in_maps.append(
            {
                "xa": xa[sl],
                "xb": xb[sl],
                "k1": k1[sl].astype(bf16),
                "k2": k2[sl].astype(bf16),
            }
        )
    return in_maps


def kernel(x, max_, min_, ycbcr_w, dct_w):
    from concourse.bass_utils import run_bass_kernel_spmd

    nc = _get_nc()
    in_maps = _make_in_maps(x, max_, min_, ycbcr_w)
    res = run_bass_kernel_spmd(nc, in_maps, core_ids=list(range(NCORES)))
    out = np.concatenate([res.results[i]["out"] for i in range(NCORES)], axis=0)
    return _untangle(out)


def _untangle(dev_out):
    """[B, p=128, g=32, 192] device layout -> [B, 192, 64, 64] f32."""
    v = np.asarray(dev_out).astype(np.float32)
    v = v.transpose(0, 2, 1, 3)  # s, g, p, (co,u,v) ; blk = g*128+p = hb*64+wb
    v = v.reshape(-1, 64, 64, 3, 8, 8)  # s, hb, wb, co, u, v
    v = v.transpose(0, 3, 4, 5, 1, 2)  # s, co, u, v, hb, wb
    return np.ascontiguousarray(v.reshape(-1, 192, 64, 64))


# revision 22
# speedup vs baseline: 1.1277x; 1.0482x over previous
"""Trainium2 Bass kernel for nn_DCT: YCbCr 3x3 channel mix + 8x8 block DCT
(stride 8) + repeated min/max normalization collapsed to a per-channel affine.

Sharding: pure data parallel, batch 32 -> 4 samples on each of 8 NeuronCores.

v3 dataflow — direct 2D DCT, everything inside one matmul stage:
  - Host shuffles x to block-pixel layout: xa[s, (ci01,i,j)=128, blk=4096],
    xb[s, (ci2,i,j)+ones=65, blk=4096] (row 64 = 1.0; blk = hb*64+wb).
  - Per-sample rhs constants carry mix, both DCTs, the affine scale AND bias:
      K[(ci,i,j),(co,u,v)] = y[co,ci]*D[u,i]*D[v,j]*s_aff[smp,co,u,v]
      K1 = K rows 0:128, K2 = K rows 128:192 + row 64 = b_aff[smp].
  - Per (s, group of 128 blocks): two accumulating matmuls
      ps[blk128, (co,u,v)] = xa_g^T @ K1 + xb_g^T @ K2   (f32 PSUM)
    give the FINAL normalized DCT output directly.
  - PSUM -> SBUF bf16 copies split between Scalar and Vector engines.
  - One contiguous 1.5MB DMA out per sample; host untangles.
"""

import math
import sys

import numpy as np

for _p in ("/opt/trn_rl_repo", "/opt/pypackages"):
    if _p not in sys.path:
        sys.path.insert(0, _p)

N = 8
IN_CH = 3
EPS = 1e-6
B_FULL = 32
H = 512
W = 512
NCORES = 8
BPC = B_FULL // NCORES  # samples per core
NBLK = 4096  # 64x64 blocks per image
NGRP = NBLK // 128  # 32 groups of 128 blocks

_CACHED_NC = None


def _dct_basis(n=N):
    u = np.arange(n)
    i = np.arange(n)
    b = np.cos(np.pi * np.outer(u, i + 0.5) / n)
    c = np.full(n, math.sqrt(2.0 / n))
    c[0] = math.sqrt(1.0 / n)
    return b * c[:, None]


def _affine_coeffs(max_, min_):
    """Closed form of t -> (t - min)/d applied B_FULL times: out = s*dct + b."""
    m = np.asarray(max_, np.float32)[..., 0, 0]
    n = np.asarray(min_, np.float32)[..., 0, 0]
    d = (m - n + np.float32(EPS)).astype(np.float64)
    r = 1.0 / d
    s = r**B_FULL
    b = -n.astype(np.float64) * (r * (1.0 - s) / (1.0 - r))
    return s, b  # [B, 192] f64


def _build_nc():
    import concourse.mybir as mybir
    import concourse.tile as tile
    from concourse import bacc
    from contextlib import ExitStack

    f32 = mybir.dt.float32
    bf16 = mybir.dt.bfloat16
    nc = bacc.Bacc()
    xa_t = nc.declare_dram_parameter("xa", [BPC, 128, NBLK], bf16, isOutput=False)
    xb_t = nc.declare_dram_parameter("xb", [BPC, 65, NBLK], bf16, isOutput=False)
    k1_t = nc.declare_dram_parameter("k1", [BPC, 128, 192], bf16, isOutput=False)
    k2_t = nc.declare_dram_parameter("k2", [BPC, 65, 192], bf16, isOutput=False)
    # out[s, p=blk%128, g=blk//128, (co,u,v)]
    out_t = nc.declare_dram_parameter("out", [BPC, 128, NGRP, 192], bf16, isOutput=True)

    with ExitStack() as ctx:
        tc = ctx.enter_context(tile.TileContext(nc))
        consts = ctx.enter_context(tc.tile_pool(name="consts", bufs=1))
        xap = ctx.enter_context(tc.tile_pool(name="xap", bufs=3))
        xbp = ctx.enter_context(tc.tile_pool(name="xbp", bufs=3))
        psp = ctx.enter_context(tc.tile_pool(name="psp", bufs=3, space="PSUM"))
        outp = ctx.enter_context(tc.tile_pool(name="outp", bufs=3))

        # queue discipline: sync issues ONLY input loads, gpsimd ONLY output
        # stores, scalar/vector only their copies — no head-of-line blocking
        # of the next sample's loads behind compute-dependent stores.
        k1_sb = consts.tile([128, BPC, 192], bf16)
        nc.sync.dma_start(out=k1_sb, in_=k1_t[:].rearrange("s k n -> k s n"))
        k2_sb = consts.tile([65, BPC, 192], bf16)
        nc.sync.dma_start(out=k2_sb, in_=k2_t[:].rearrange("s k n -> k s n"))

        for s in range(BPC):
            xa = xap.tile([128, NBLK], bf16)
            nc.sync.dma_start(out=xa, in_=xa_t[s])
            xb = xbp.tile([65, NBLK], bf16)
            nc.sync.dma_start(out=xb, in_=xb_t[s])
            for half in range(2):
                stg = outp.tile([128, 16, 192], bf16)
                for t4h in range(4):  # 4 PSUM tiles x 4 groups per half
                    t4 = half * 4 + t4h
                    ps = psp.tile([128, 4, 256], f32)  # 2 banks; [:, q, 0:192]
                    for q in range(4):
                        g = t4 * 4 + q
                        nc.tensor.matmul(
                            ps[:, q, 0:192],
                            lhsT=xa[:, g * 128 : (g + 1) * 128],
                            rhs=k1_sb[:, s],
                            start=True,
                            stop=False,
                        )
                        nc.tensor.matmul(
                            ps[:, q, 0:192],
                            lhsT=xb[:, g * 128 : (g + 1) * 128],
                            rhs=k2_sb[:, s],
                            start=False,
                            stop=True,
                        )
                    dst = stg[:, t4h * 4 : (t4h + 1) * 4]
                    if t4 % 2 == 0:
                        nc.scalar.copy(out=dst, in_=ps[:, :, 0:192])
                    else:
                        nc.vector.tensor_copy(out=dst, in_=ps[:, :, 0:192])
                # one 6KB-run store per 16 groups, on the dedicated out queue
                nc.gpsimd.dma_start(
                    out=out_t[s][:, half * 16 : (half + 1) * 16], in_=stg
                )
    return nc


def _get_nc():
    global _CACHED_NC
    if _CACHED_NC is None:
        nc = _build_nc()
        if not nc.is_finalized():
            nc.finalize()
        _CACHED_NC = nc
    return _CACHED_NC


def _make_in_maps(x, max_, min_, ycbcr_w):
    import ml_dtypes

    bf16 = ml_dtypes.bfloat16
    x16 = np.asarray(x, np.float32).astype(bf16)
    # block-pixel layout: [B, (ci,i,j)=192, blk=(hb,wb)=4096]
    xd = x16.reshape(-1, 3, 64, 8, 64, 8)  # s, ci, hb, i, wb, j
    xd = np.ascontiguousarray(xd.transpose(0, 1, 3, 5, 2, 4))  # s, ci, i, j, hb, wb
    xd = xd.reshape(-1, 192, NBLK)
    ones = np.ones((xd.shape[0], 1, NBLK), bf16)
    xa = np.ascontiguousarray(xd[:, 0:128])
    xb = np.ascontiguousarray(np.concatenate([xd[:, 128:192], ones], axis=1))

    s_aff, b_aff = _affine_coeffs(max_, min_)  # [B, 192] f64 (co,u,v)
    D = _dct_basis()  # [u, i] f64
    y = np.asarray(ycbcr_w, np.float64)  # [co, ci]
    # K[(ci,i,j), (co,u,v)] * s_aff[smp, (co,u,v)]
    kbase = np.einsum("oc,ui,vj->cijouv", y, D, D).reshape(192, 192)
    ks = kbase[None, :, :] * s_aff[:, None, :]  # [B, 192, 192]
    k1 = ks[:, 0:128]
    k2 = np.concatenate([ks[:, 128:192], b_aff[:, None, :]], axis=1)  # [B, 65, 192]

    in_maps = []
    for core in range(NCORES):
        sl = slice(core * BPC, (core + 1) * BPC)
        in_maps.append(
            {
                "xa": xa[sl],
                "xb": xb[sl],
                "k1": k1[sl].astype(bf16),
                "k2": k2[sl].astype(bf16),
            }
        )
    return in_maps


def kernel(x, max_, min_, ycbcr_w, dct_w):
    from concourse.bass_utils import run_bass_kernel_spmd

    nc = _get_nc()
    in_maps = _make_in_maps(x, max_, min_, ycbcr_w)
    res = run_bass_kernel_spmd(nc, in_maps, core_ids=list(range(NCORES)))
    out = np.concatenate([res.results[i]["out"] for i in range(NCORES)], axis=0)
    return _untangle(out)


def _untangle(dev_out):
    """[B, p=128, g=32, 192] device layout -> [B, 192, 64, 64] f32."""
    v = np.asarray(dev_out).astype(np.float32)
    v = v.transpose(0, 2, 1, 3)  # s, g, p, (co,u,v) ; blk = g*128+p = hb*64+wb
    v = v.reshape(-1, 64, 64, 3, 8, 8)  # s, hb, wb, co, u, v
    v = v.transpose(0, 3, 4, 5, 1, 2)  # s, co, u, v, hb, wb
    return np.ascontiguousarray(v.reshape(-1, 192, 64, 64))


# revision 23
# speedup vs baseline: 1.2071x; 1.0704x over previous
"""Trainium2 Bass kernel for nn_DCT: YCbCr 3x3 channel mix + 8x8 block DCT
(stride 8) + repeated min/max normalization collapsed to a per-channel affine.

Sharding: pure data parallel, batch 32 -> 4 samples on each of 8 NeuronCores.

v3 dataflow — direct 2D DCT, everything inside one matmul stage:
  - Host shuffles x to block-pixel layout: xa[s, (ci01,i,j)=128, blk=4096],
    xb[s, (ci2,i,j)+ones=65, blk=4096] (row 64 = 1.0; blk = hb*64+wb).
  - Per-sample rhs constants carry mix, both DCTs, the affine scale AND bias:
      K[(ci,i,j),(co,u,v)] = y[co,ci]*D[u,i]*D[v,j]*s_aff[smp,co,u,v]
      K1 = K rows 0:128, K2 = K rows 128:192 + row 64 = b_aff[smp].
  - Per (s, group of 128 blocks): two accumulating matmuls
      ps[blk128, (co,u,v)] = xa_g^T @ K1 + xb_g^T @ K2   (f32 PSUM)
    give the FINAL normalized DCT output directly.
  - PSUM -> SBUF bf16 copies split between Scalar and Vector engines.
  - One contiguous 1.5MB DMA out per sample; host untangles.
"""

import math
import sys

import numpy as np

for _p in ("/opt/trn_rl_repo", "/opt/pypackages"):
    if _p not in sys.path:
        sys.path.insert(0, _p)

N = 8
IN_CH = 3
EPS = 1e-6
B_FULL = 32
H = 512
W = 512
NCORES = 8
BPC = B_FULL // NCORES  # samples per core
NBLK = 4096  # 64x64 blocks per image
NGRP = NBLK // 128  # 32 groups of 128 blocks

_CACHED_NC = None


def _dct_basis(n=N):
    u = np.arange(n)
    i = np.arange(n)
    b = np.cos(np.pi * np.outer(u, i + 0.5) / n)
    c = np.full(n, math.sqrt(2.0 / n))
    c[0] = math.sqrt(1.0 / n)
    return b * c[:, None]


def _affine_coeffs(max_, min_):
    """Closed form of t -> (t - min)/d applied B_FULL times: out = s*dct + b."""
    m = np.asarray(max_, np.float32)[..., 0, 0]
    n = np.asarray(min_, np.float32)[..., 0, 0]
    d = (m - n + np.float32(EPS)).astype(np.float64)
    r = 1.0 / d
    s = r**B_FULL
    b = -n.astype(np.float64) * (r * (1.0 - s) / (1.0 - r))
    return s, b  # [B, 192] f64


def _build_nc():
    import concourse.mybir as mybir
    import concourse.tile as tile
    from concourse import bacc
    from contextlib import ExitStack

    f32 = mybir.dt.float32
    bf16 = mybir.dt.bfloat16
    nc = bacc.Bacc()
    xa_t = nc.declare_dram_parameter("xa", [BPC, 128, NBLK], bf16, isOutput=False)
    xb_t = nc.declare_dram_parameter("xb", [BPC, 65, NBLK], bf16, isOutput=False)
    k1_t = nc.declare_dram_parameter("k1", [BPC, 128, 192], bf16, isOutput=False)
    k2_t = nc.declare_dram_parameter("k2", [BPC, 65, 192], bf16, isOutput=False)
    # out[s, p=blk%128, g=blk//128, (co,u,v)]
    out_t = nc.declare_dram_parameter("out", [BPC, 128, NGRP, 192], bf16, isOutput=True)

    with ExitStack() as ctx:
        tc = ctx.enter_context(tile.TileContext(nc))
        consts = ctx.enter_context(tc.tile_pool(name="consts", bufs=1))
        xap = ctx.enter_context(tc.tile_pool(name="xap", bufs=3))
        xbp = ctx.enter_context(tc.tile_pool(name="xbp", bufs=3))
        psp = ctx.enter_context(tc.tile_pool(name="psp", bufs=4, space="PSUM"))
        outp = ctx.enter_context(tc.tile_pool(name="outp", bufs=4))

        # queue discipline: sync issues ONLY input loads, gpsimd ONLY output
        # stores, scalar/vector only their copies — no head-of-line blocking
        # of the next sample's loads behind compute-dependent stores.
        k1_sb = consts.tile([128, BPC, 192], bf16)
        nc.sync.dma_start(out=k1_sb, in_=k1_t[:].rearrange("s k n -> k s n"))
        k2_sb = consts.tile([65, BPC, 192], bf16)
        nc.sync.dma_start(out=k2_sb, in_=k2_t[:].rearrange("s k n -> k s n"))

        for s in range(BPC):
            xa = xap.tile([128, NBLK], bf16)
            nc.sync.dma_start(out=xa, in_=xa_t[s])
            xb = xbp.tile([65, NBLK], bf16)
            nc.sync.dma_start(out=xb, in_=xb_t[s])
            for half in range(2):
                stg = outp.tile([128, 16, 192], bf16)
                for t4h in range(4):  # 4 PSUM tiles x 4 groups per half
                    t4 = half * 4 + t4h
                    ps = psp.tile([128, 4, 256], f32)  # 2 banks; [:, q, 0:192]
                    for q in range(4):
                        g = t4 * 4 + q
                        nc.tensor.matmul(
                            ps[:, q, 0:192],
                            lhsT=xa[:, g * 128 : (g + 1) * 128],
                            rhs=k1_sb[:, s],
                            start=True,
                            stop=False,
                        )
                        nc.tensor.matmul(
                            ps[:, q, 0:192],
                            lhsT=xb[:, g * 128 : (g + 1) * 128],
                            rhs=k2_sb[:, s],
                            start=False,
                            stop=True,
                        )
                    dst = stg[:, t4h * 4 : (t4h + 1) * 4]
                    if t4 % 2 == 0:
                        nc.scalar.copy(out=dst, in_=ps[:, :, 0:192])
                    else:
                        nc.vector.tensor_copy(out=dst, in_=ps[:, :, 0:192])
                # one 6KB-run store per 16 groups, on the dedicated out queue
                nc.gpsimd.dma_start(
                    out=out_t[s][:, half * 16 : (half + 1) * 16], in_=stg
                )
    return nc


def _get_nc():
    global _CACHED_NC
    if _CACHED_NC is None:
        nc = _build_nc()
        if not nc.is_finalized():
            nc.finalize()
        _CACHED_NC = nc
    return _CACHED_NC


def _make_in_maps(x, max_, min_, ycbcr_w):
    import ml_dtypes

    bf16 = ml_dtypes.bfloat16
    x16 = np.asarray(x, np.float32).astype(bf16)
    # block-pixel layout: [B, (ci,i,j)=192, blk=(hb,wb)=4096]
    xd = x16.reshape(-1, 3, 64, 8, 64, 8)  # s, ci, hb, i, wb, j
    xd = np.ascontiguousarray(xd.transpose(0, 1, 3, 5, 2, 4))  # s, ci, i, j, hb, wb
    xd = xd.reshape(-1, 192, NBLK)
    ones = np.ones((xd.shape[0], 1, NBLK), bf16)
    xa = np.ascontiguousarray(xd[:, 0:128])
    xb = np.ascontiguousarray(np.concatenate([xd[:, 128:192], ones], axis=1))

    s_aff, b_aff = _affine_coeffs(max_, min_)  # [B, 192] f64 (co,u,v)
    D = _dct_basis()  # [u, i] f64
    y = np.asarray(ycbcr_w, np.float64)  # [co, ci]
    # K[(ci,i,j), (co,u,v)] * s_aff[smp, (co,u,v)]
    kbase = np.einsum("oc,ui,vj->cijouv", y, D, D).reshape(192, 192)
    ks = kbase[None, :, :] * s_aff[:, None, :]  # [B, 192, 192]
    k1 = ks[:, 0:128]
    k2 = np.concatenate([ks[:, 128:192], b_aff[:, None, :]], axis=1)  # [B, 65, 192]

    in_maps = []
    for core in range(NCORES):
        sl = slice(core * BPC, (core + 1) * BPC)
        in_maps.append(
            {
                "xa": xa[sl],
                "xb": xb[sl],
                "k1": k1[sl].astype(bf16),
                "k2": k2[sl].astype(bf16),
            }
        )
    return in_maps


def kernel(x, max_, min_, ycbcr_w, dct_w):
    from concourse.bass_utils import run_bass_kernel_spmd

    nc = _get_nc()
    in_maps = _make_in_maps(x, max_, min_, ycbcr_w)
    res = run_bass_kernel_spmd(nc, in_maps, core_ids=list(range(NCORES)))
    out = np.concatenate([res.results[i]["out"] for i in range(NCORES)], axis=0)
    return _untangle(out)


def _untangle(dev_out):
    """[B, p=128, g=32, 192] device layout -> [B, 192, 64, 64] f32."""
    v = np.asarray(dev_out).astype(np.float32)
    v = v.transpose(0, 2, 1, 3)  # s, g, p, (co,u,v) ; blk = g*128+p = hb*64+wb
    v = v.reshape(-1, 64, 64, 3, 8, 8)  # s, hb, wb, co, u, v
    v = v.transpose(0, 3, 4, 5, 1, 2)  # s, co, u, v, hb, wb
    return np.ascontiguousarray(v.reshape(-1, 192, 64, 64))


# revision 25
# speedup vs baseline: 1.2267x; 1.0163x over previous
"""Trainium2 Bass kernel for nn_DCT: YCbCr 3x3 channel mix + 8x8 block DCT
(stride 8) + repeated min/max normalization collapsed to a per-channel affine.

Sharding: pure data parallel, batch 32 -> 4 samples on each of 8 NeuronCores.

v3 dataflow — direct 2D DCT, everything inside one matmul stage:
  - Host shuffles x to block-pixel layout: xa[s, (ci01,i,j)=128, blk=4096],
    xb[s, (ci2,i,j)+ones=65, blk=4096] (row 64 = 1.0; blk = hb*64+wb).
  - Per-sample rhs constants carry mix, both DCTs, the affine scale AND bias:
      K[(ci,i,j),(co,u,v)] = y[co,ci]*D[u,i]*D[v,j]*s_aff[smp,co,u,v]
      K1 = K rows 0:128, K2 = K rows 128:192 + row 64 = b_aff[smp].
  - Per (s, group of 128 blocks): two accumulating matmuls
      ps[blk128, (co,u,v)] = xa_g^T @ K1 + xb_g^T @ K2   (f32 PSUM)
    give the FINAL normalized DCT output directly.
  - PSUM -> SBUF bf16 copies split between Scalar and Vector engines.
  - One contiguous 1.5MB DMA out per sample; host untangles.
"""

import math
import sys

import numpy as np

for _p in ("/opt/trn_rl_repo", "/opt/pypackages"):
    if _p not in sys.path:
        sys.path.insert(0, _p)

N = 8
IN_CH = 3
EPS = 1e-6
B_FULL = 32
H = 512
W = 512
NCORES = 8
BPC = B_FULL // NCORES  # samples per core
NBLK = 4096  # 64x64 blocks per image
NGRP = NBLK // 128  # 32 groups of 128 blocks

_CACHED_NC = None


def _dct_basis(n=N):
    u = np.arange(n)
    i = np.arange(n)
    b = np.cos(np.pi * np.outer(u, i + 0.5) / n)
    c = np.full(n, math.sqrt(2.0 / n))
    c[0] = math.sqrt(1.0 / n)
    return b * c[:, None]


def _affine_coeffs(max_, min_):
    """Closed form of t -> (t - min)/d applied B_FULL times: out = s*dct + b."""
    m = np.asarray(max_, np.float32)[..., 0, 0]
    n = np.asarray(min_, np.float32)[..., 0, 0]
    d = (m - n + np.float32(EPS)).astype(np.float64)
    r = 1.0 / d
    s = r**B_FULL
    b = -n.astype(np.float64) * (r * (1.0 - s) / (1.0 - r))
    return s, b  # [B, 192] f64


def _build_nc():
    import concourse.mybir as mybir
    import concourse.tile as tile
    from concourse import bacc
    from contextlib import ExitStack

    f32 = mybir.dt.float32
    bf16 = mybir.dt.bfloat16
    nc = bacc.Bacc()
    xa_t = nc.declare_dram_parameter("xa", [BPC, 128, NBLK], bf16, isOutput=False)
    xb_t = nc.declare_dram_parameter("xb", [BPC, 65, NBLK], bf16, isOutput=False)
    k1_t = nc.declare_dram_parameter("k1", [BPC, 128, 192], bf16, isOutput=False)
    k2_t = nc.declare_dram_parameter("k2", [BPC, 65, 192], bf16, isOutput=False)
    # out[s, p=blk%128, g=blk//128, (co,u,v)]
    out_t = nc.declare_dram_parameter("out", [BPC, 128, NGRP, 192], bf16, isOutput=True)

    with ExitStack() as ctx:
        tc = ctx.enter_context(tile.TileContext(nc))
        consts = ctx.enter_context(tc.tile_pool(name="consts", bufs=1))
        xap = ctx.enter_context(tc.tile_pool(name="xap", bufs=3))
        xbp = ctx.enter_context(tc.tile_pool(name="xbp", bufs=3))
        psp = ctx.enter_context(tc.tile_pool(name="psp", bufs=4, space="PSUM"))
        outp = ctx.enter_context(tc.tile_pool(name="outp", bufs=8))

        # queue discipline: sync issues ONLY input loads, gpsimd ONLY output
        # stores, scalar/vector only their copies — no head-of-line blocking
        # of the next sample's loads behind compute-dependent stores.
        k1_sb = consts.tile([128, BPC, 192], bf16)
        nc.sync.dma_start(out=k1_sb, in_=k1_t[:].rearrange("s k n -> k s n"))
        k2_sb = consts.tile([65, BPC, 192], bf16)
        nc.sync.dma_start(out=k2_sb, in_=k2_t[:].rearrange("s k n -> k s n"))

        for s in range(BPC):
            xa = xap.tile([128, NBLK], bf16)
            nc.sync.dma_start(out=xa, in_=xa_t[s])
            xb = xbp.tile([65, NBLK], bf16)
            nc.sync.dma_start(out=xb, in_=xb_t[s])
            for half in range(4):
                stg = outp.tile([128, 8, 192], bf16)
                for t4h in range(2):  # 2 PSUM tiles x 4 groups per store
                    t4 = half * 2 + t4h
                    ps = psp.tile([128, 4, 256], f32)  # 2 banks; [:, q, 0:192]
                    for q in range(4):
                        g = t4 * 4 + q
                        nc.tensor.matmul(
                            ps[:, q, 0:192],
                            lhsT=xa[:, g * 128 : (g + 1) * 128],
                            rhs=k1_sb[:, s],
                            start=True,
                            stop=False,
                        )
                        nc.tensor.matmul(
                            ps[:, q, 0:192],
                            lhsT=xb[:, g * 128 : (g + 1) * 128],
                            rhs=k2_sb[:, s],
                            start=False,
                            stop=True,
                        )
                    dst = stg[:, t4h * 4 : (t4h + 1) * 4]
                    if t4 % 2 == 0:
                        nc.scalar.copy(out=dst, in_=ps[:, :, 0:192])
                    else:
                        nc.vector.tensor_copy(out=dst, in_=ps[:, :, 0:192])
                # 3KB-run store per 8 groups, on the dedicated out queue
                nc.gpsimd.dma_start(
                    out=out_t[s][:, half * 8 : (half + 1) * 8], in_=stg
                )
    return nc


def _get_nc():
    global _CACHED_NC
    if _CACHED_NC is None:
        nc = _build_nc()
        if not nc.is_finalized():
            nc.finalize()
        _CACHED_NC = nc
    return _CACHED_NC


def _make_in_maps(x, max_, min_, ycbcr_w):
    import ml_dtypes

    bf16 = ml_dtypes.bfloat16
    x16 = np.asarray(x, np.float32).astype(bf16)
    # block-pixel layout: [B, (ci,i,j)=192, blk=(hb,wb)=4096]
    xd = x16.reshape(-1, 3, 64, 8, 64, 8)  # s, ci, hb, i, wb, j
    xd = np.ascontiguousarray(xd.transpose(0, 1, 3, 5, 2, 4))  # s, ci, i, j, hb, wb
    xd = xd.reshape(-1, 192, NBLK)
    ones = np.ones((xd.shape[0], 1, NBLK), bf16)
    xa = np.ascontiguousarray(xd[:, 0:128])
    xb = np.ascontiguousarray(np.concatenate([xd[:, 128:192], ones], axis=1))

    s_aff, b_aff = _affine_coeffs(max_, min_)  # [B, 192] f64 (co,u,v)
    D = _dct_basis()  # [u, i] f64
    y = np.asarray(ycbcr_w, np.float64)  # [co, ci]
    # K[(ci,i,j), (co,u,v)] * s_aff[smp, (co,u,v)]
    kbase = np.einsum("oc,ui,vj->cijouv", y, D, D).reshape(192, 192)
    ks = kbase[None, :, :] * s_aff[:, None, :]  # [B, 192, 192]
    k1 = ks[:, 0:128]
    k2 = np.concatenate([ks[:, 128:192], b_aff[:, None, :]], axis=1)  # [B, 65, 192]

    in_maps = []
    for core in range(NCORES):
        sl = slice(core * BPC, (core + 1) * BPC)
        in_maps.append(
            {
                "xa": xa[sl],
                "xb": xb[sl],
                "k1": k1[sl].astype(bf16),
                "k2": k2[sl].astype(bf16),
            }
        )
    return in_maps


def kernel(x, max_, min_, ycbcr_w, dct_w):
    from concourse.bass_utils import run_bass_kernel_spmd

    nc = _get_nc()
    in_maps = _make_in_maps(x, max_, min_, ycbcr_w)
    res = run_bass_kernel_spmd(nc, in_maps, core_ids=list(range(NCORES)))
    out = np.concatenate([res.results[i]["out"] for i in range(NCORES)], axis=0)
    return _untangle(out)


def _untangle(dev_out):
    """[B, p=128, g=32, 192] device layout -> [B, 192, 64, 64] f32."""
    v = np.asarray(dev_out).astype(np.float32)
    v = v.transpose(0, 2, 1, 3)  # s, g, p, (co,u,v) ; blk = g*128+p = hb*64+wb
    v = v.reshape(-1, 64, 64, 3, 8, 8)  # s, hb, wb, co, u, v
    v = v.transpose(0, 3, 4, 5, 1, 2)  # s, co, u, v, hb, wb
    return np.ascontiguousarray(v.reshape(-1, 192, 64, 64))


# revision 26
# speedup vs baseline: 1.2478x; 1.0172x over previous
"""Trainium2 Bass kernel for nn_DCT: YCbCr 3x3 channel mix + 8x8 block DCT
(stride 8) + repeated min/max normalization collapsed to a per-channel affine.

Sharding: pure data parallel, batch 32 -> 4 samples on each of 8 NeuronCores.

v3 dataflow — direct 2D DCT, everything inside one matmul stage:
  - Host shuffles x to block-pixel layout: xa[s, (ci01,i,j)=128, blk=4096],
    xb[s, (ci2,i,j)+ones=65, blk=4096] (row 64 = 1.0; blk = hb*64+wb).
  - Per-sample rhs constants carry mix, both DCTs, the affine scale AND bias:
      K[(ci,i,j),(co,u,v)] = y[co,ci]*D[u,i]*D[v,j]*s_aff[smp,co,u,v]
      K1 = K rows 0:128, K2 = K rows 128:192 + row 64 = b_aff[smp].
  - Per (s, group of 128 blocks): two accumulating matmuls
      ps[blk128, (co,u,v)] = xa_g^T @ K1 + xb_g^T @ K2   (f32 PSUM)
    give the FINAL normalized DCT output directly.
  - PSUM -> SBUF bf16 copies split between Scalar and Vector engines.
  - One contiguous 1.5MB DMA out per sample; host untangles.
"""

import math
import sys

import numpy as np

for _p in ("/opt/trn_rl_repo", "/opt/pypackages"):
    if _p not in sys.path:
        sys.path.insert(0, _p)

N = 8
IN_CH = 3
EPS = 1e-6
B_FULL = 32
H = 512
W = 512
NCORES = 8
BPC = B_FULL // NCORES  # samples per core
NBLK = 4096  # 64x64 blocks per image
NGRP = NBLK // 128  # 32 groups of 128 blocks

_CACHED_NC = None


def _dct_basis(n=N):
    u = np.arange(n)
    i = np.arange(n)
    b = np.cos(np.pi * np.outer(u, i + 0.5) / n)
    c = np.full(n, math.sqrt(2.0 / n))
    c[0] = math.sqrt(1.0 / n)
    return b * c[:, None]


def _affine_coeffs(max_, min_):
    """Closed form of t -> (t - min)/d applied B_FULL times: out = s*dct + b."""
    m = np.asarray(max_, np.float32)[..., 0, 0]
    n = np.asarray(min_, np.float32)[..., 0, 0]
    d = (m - n + np.float32(EPS)).astype(np.float64)
    r = 1.0 / d
    s = r**B_FULL
    b = -n.astype(np.float64) * (r * (1.0 - s) / (1.0 - r))
    return s, b  # [B, 192] f64


def _build_nc():
    import concourse.mybir as mybir
    import concourse.tile as tile
    from concourse import bacc
    from contextlib import ExitStack

    f32 = mybir.dt.float32
    bf16 = mybir.dt.bfloat16
    nc = bacc.Bacc()
    xa_t = nc.declare_dram_parameter("xa", [BPC, 128, NBLK], bf16, isOutput=False)
    xb_t = nc.declare_dram_parameter("xb", [BPC, 65, NBLK], bf16, isOutput=False)
    k1_t = nc.declare_dram_parameter("k1", [BPC, 128, 192], bf16, isOutput=False)
    k2_t = nc.declare_dram_parameter("k2", [BPC, 65, 192], bf16, isOutput=False)
    # out[s, p=blk%128, g=blk//128, (co,u,v)]
    out_t = nc.declare_dram_parameter("out", [BPC, 128, NGRP, 192], bf16, isOutput=True)

    with ExitStack() as ctx:
        tc = ctx.enter_context(tile.TileContext(nc))
        consts = ctx.enter_context(tc.tile_pool(name="consts", bufs=1))
        xap = ctx.enter_context(tc.tile_pool(name="xap", bufs=3))
        xbp = ctx.enter_context(tc.tile_pool(name="xbp", bufs=3))
        psp = ctx.enter_context(tc.tile_pool(name="psp", bufs=4, space="PSUM"))
        outp = ctx.enter_context(tc.tile_pool(name="outp", bufs=8))

        # queue discipline: sync issues ONLY input loads, gpsimd ONLY output
        # stores, scalar/vector only their copies — no head-of-line blocking
        # of the next sample's loads behind compute-dependent stores.
        k1_sb = consts.tile([128, BPC, 192], bf16)
        nc.sync.dma_start(out=k1_sb, in_=k1_t[:].rearrange("s k n -> k s n"))
        k2_sb = consts.tile([65, BPC, 192], bf16)
        nc.sync.dma_start(out=k2_sb, in_=k2_t[:].rearrange("s k n -> k s n"))

        for s in range(BPC):
            # half-tile loads: the first 16 groups' matmuls depend only on the
            # first halves, so compute starts ~2.5us into the sample-0 load
            xa = xap.tile([128, NBLK], bf16)
            xb = xbp.tile([65, NBLK], bf16)
            for hq in range(2):
                sl = slice(hq * 2048, (hq + 1) * 2048)
                nc.sync.dma_start(out=xa[:, sl], in_=xa_t[s][:, sl])
                nc.sync.dma_start(out=xb[:, sl], in_=xb_t[s][:, sl])
            for half in range(4):
                stg = outp.tile([128, 8, 192], bf16)
                for t4h in range(2):  # 2 PSUM tiles x 4 groups per store
                    t4 = half * 2 + t4h
                    ps = psp.tile([128, 4, 256], f32)  # 2 banks; [:, q, 0:192]
                    for q in range(4):
                        g = t4 * 4 + q
                        nc.tensor.matmul(
                            ps[:, q, 0:192],
                            lhsT=xa[:, g * 128 : (g + 1) * 128],
                            rhs=k1_sb[:, s],
                            start=True,
                            stop=False,
                        )
                        nc.tensor.matmul(
                            ps[:, q, 0:192],
                            lhsT=xb[:, g * 128 : (g + 1) * 128],
                            rhs=k2_sb[:, s],
                            start=False,
                            stop=True,
                        )
                    dst = stg[:, t4h * 4 : (t4h + 1) * 4]
                    if t4 % 2 == 0:
                        nc.scalar.copy(out=dst, in_=ps[:, :, 0:192])
                    else:
                        nc.vector.tensor_copy(out=dst, in_=ps[:, :, 0:192])
                # 3KB-run store per 8 groups, on the dedicated out queue
                nc.gpsimd.dma_start(
                    out=out_t[s][:, half * 8 : (half + 1) * 8], in_=stg
                )
    return nc


def _get_nc():
    global _CACHED_NC
    if _CACHED_NC is None:
        nc = _build_nc()
        if not nc.is_finalized():
            nc.finalize()
        _CACHED_NC = nc
    return _CACHED_NC


def _make_in_maps(x, max_, min_, ycbcr_w):
    import ml_dtypes

    bf16 = ml_dtypes.bfloat16
    x16 = np.asarray(x, np.float32).astype(bf16)
    # block-pixel layout: [B, (ci,i,j)=192, blk=(hb,wb)=4096]
    xd = x16.reshape(-1, 3, 64, 8, 64, 8)  # s, ci, hb, i, wb, j
    xd = np.ascontiguousarray(xd.transpose(0, 1, 3, 5, 2, 4))  # s, ci, i, j, hb, wb
    xd = xd.reshape(-1, 192, NBLK)
    ones = np.ones((xd.shape[0], 1, NBLK), bf16)
    xa = np.ascontiguousarray(xd[:, 0:128])
    xb = np.ascontiguousarray(np.concatenate([xd[:, 128:192], ones], axis=1))

    s_aff, b_aff = _affine_coeffs(max_, min_)  # [B, 192] f64 (co,u,v)
    D = _dct_basis()  # [u, i] f64
    y = np.asarray(ycbcr_w, np.float64)  # [co, ci]
    # K[(ci,i,j), (co,u,v)] * s_aff[smp, (co,u,v)]
    kbase = np.einsum("oc,ui,vj->cijouv", y, D, D).reshape(192, 192)
    ks = kbase[None, :, :] * s_aff[:, None, :]  # [B, 192, 192]
    k1 = ks[:, 0:128]
    k2 = np.concatenate([ks[:, 128:192], b_aff[:, None, :]], axis=1)  # [B, 65, 192]

    in_maps = []
    for core in range(NCORES):
        sl = slice(core * BPC, (core + 1) * BPC)
        in_maps.append(
            {
                "xa": xa[sl],
                "xb": xb[sl],
                "k1": k1[sl].astype(bf16),
                "k2": k2[sl].astype(bf16),
            }
        )
    return in_maps


def kernel(x, max_, min_, ycbcr_w, dct_w):
    from concourse.bass_utils import run_bass_kernel_spmd

    nc = _get_nc()
    in_maps = _make_in_maps(x, max_, min_, ycbcr_w)
    res = run_bass_kernel_spmd(nc, in_maps, core_ids=list(range(NCORES)))
    out = np.concatenate([res.results[i]["out"] for i in range(NCORES)], axis=0)
    return _untangle(out)


def _untangle(dev_out):
    """[B, p=128, g=32, 192] device layout -> [B, 192, 64, 64] f32."""
    v = np.asarray(dev_out).astype(np.float32)
    v = v.transpose(0, 2, 1, 3)  # s, g, p, (co,u,v) ; blk = g*128+p = hb*64+wb
    v = v.reshape(-1, 64, 64, 3, 8, 8)  # s, hb, wb, co, u, v
    v = v.transpose(0, 3, 4, 5, 1, 2)  # s, co, u, v, hb, wb
    return np.ascontiguousarray(v.reshape(-1, 192, 64, 64))


# revision 28
# speedup vs baseline: 1.2758x; 1.0225x over previous
"""Trainium2 Bass kernel for nn_DCT: YCbCr 3x3 channel mix + 8x8 block DCT
(stride 8) + repeated min/max normalization collapsed to a per-channel affine.

Sharding: pure data parallel, batch 32 -> 4 samples on each of 8 NeuronCores.

v3 dataflow — direct 2D DCT, everything inside one matmul stage:
  - Host shuffles x to block-pixel layout: xa[s, (ci01,i,j)=128, blk=4096],
    xb[s, (ci2,i,j)+ones=65, blk=4096] (row 64 = 1.0; blk = hb*64+wb).
  - Per-sample rhs constants carry mix, both DCTs, the affine scale AND bias:
      K[(ci,i,j),(co,u,v)] = y[co,ci]*D[u,i]*D[v,j]*s_aff[smp,co,u,v]
      K1 = K rows 0:128, K2 = K rows 128:192 + row 64 = b_aff[smp].
  - Per (s, group of 128 blocks): two accumulating matmuls
      ps[blk128, (co,u,v)] = xa_g^T @ K1 + xb_g^T @ K2   (f32 PSUM)
    give the FINAL normalized DCT output directly.
  - PSUM -> SBUF bf16 copies split between Scalar and Vector engines.
  - One contiguous 1.5MB DMA out per sample; host untangles.
"""

import math
import sys

import numpy as np

for _p in ("/opt/trn_rl_repo", "/opt/pypackages"):
    if _p not in sys.path:
        sys.path.insert(0, _p)

N = 8
IN_CH = 3
EPS = 1e-6
B_FULL = 32
H = 512
W = 512
NCORES = 8
BPC = B_FULL // NCORES  # samples per core
NBLK = 4096  # 64x64 blocks per image
NGRP = NBLK // 128  # 32 groups of 128 blocks

_CACHED_NC = None


def _dct_basis(n=N):
    u = np.arange(n)
    i = np.arange(n)
    b = np.cos(np.pi * np.outer(u, i + 0.5) / n)
    c = np.full(n, math.sqrt(2.0 / n))
    c[0] = math.sqrt(1.0 / n)
    return b * c[:, None]


def _affine_coeffs(max_, min_):
    """Closed form of t -> (t - min)/d applied B_FULL times: out = s*dct + b."""
    m = np.asarray(max_, np.float32)[..., 0, 0]
    n = np.asarray(min_, np.float32)[..., 0, 0]
    d = (m - n + np.float32(EPS)).astype(np.float64)
    r = 1.0 / d
    s = r**B_FULL
    b = -n.astype(np.float64) * (r * (1.0 - s) / (1.0 - r))
    return s, b  # [B, 192] f64


def _build_nc():
    import concourse.mybir as mybir
    import concourse.tile as tile
    from concourse import bacc
    from contextlib import ExitStack

    f32 = mybir.dt.float32
    bf16 = mybir.dt.bfloat16
    nc = bacc.Bacc()
    xa_t = nc.declare_dram_parameter("xa", [BPC, 128, NBLK], bf16, isOutput=False)
    xb_t = nc.declare_dram_parameter("xb", [BPC, 65, NBLK], bf16, isOutput=False)
    k1_t = nc.declare_dram_parameter("k1", [BPC, 128, 192], bf16, isOutput=False)
    k2_t = nc.declare_dram_parameter("k2", [BPC, 65, 192], bf16, isOutput=False)
    # out[s, p=blk%128, g=blk//128, (co,u,v)]
    out_t = nc.declare_dram_parameter("out", [BPC, 128, NGRP, 192], bf16, isOutput=True)

    with ExitStack() as ctx:
        tc = ctx.enter_context(tile.TileContext(nc))
        consts = ctx.enter_context(tc.tile_pool(name="consts", bufs=1))
        xap = ctx.enter_context(tc.tile_pool(name="xap", bufs=4))
        xbp = ctx.enter_context(tc.tile_pool(name="xbp", bufs=4))
        psp = ctx.enter_context(tc.tile_pool(name="psp", bufs=4, space="PSUM"))
        outp = ctx.enter_context(tc.tile_pool(name="outp", bufs=12))

        # queue discipline: sync issues ONLY input loads, gpsimd ONLY output
        # stores, scalar/vector only their copies — no head-of-line blocking
        # of the next sample's loads behind compute-dependent stores.
        k1_sb = consts.tile([128, BPC, 192], bf16)
        nc.sync.dma_start(out=k1_sb, in_=k1_t[:].rearrange("s k n -> k s n"))
        k2_sb = consts.tile([65, BPC, 192], bf16)
        nc.sync.dma_start(out=k2_sb, in_=k2_t[:].rearrange("s k n -> k s n"))

        for s in range(BPC):
            xa = xap.tile([128, NBLK], bf16)
            nc.sync.dma_start(out=xa, in_=xa_t[s])
            xb = xbp.tile([65, NBLK], bf16)
            nc.sync.dma_start(out=xb, in_=xb_t[s])
            for half in range(4):
                stg = outp.tile([128, 8, 192], bf16)
                for t4h in range(2):  # 2 PSUM tiles x 4 groups per store
                    t4 = half * 2 + t4h
                    ps = psp.tile([128, 4, 256], f32)  # 2 banks; [:, q, 0:192]
                    for q in range(4):
                        g = t4 * 4 + q
                        nc.tensor.matmul(
                            ps[:, q, 0:192],
                            lhsT=xa[:, g * 128 : (g + 1) * 128],
                            rhs=k1_sb[:, s],
                            start=True,
                            stop=False,
                        )
                        nc.tensor.matmul(
                            ps[:, q, 0:192],
                            lhsT=xb[:, g * 128 : (g + 1) * 128],
                            rhs=k2_sb[:, s],
                            start=False,
                            stop=True,
                        )
                    dst = stg[:, t4h * 4 : (t4h + 1) * 4]
                    if t4 % 2 == 0:
                        nc.scalar.copy(out=dst, in_=ps[:, :, 0:192])
                    else:
                        nc.vector.tensor_copy(out=dst, in_=ps[:, :, 0:192])
                # 3KB-run store per 8 groups, on the dedicated out queue
                nc.gpsimd.dma_start(
                    out=out_t[s][:, half * 8 : (half + 1) * 8], in_=stg
                )
    return nc


def _get_nc():
    global _CACHED_NC
    if _CACHED_NC is None:
        nc = _build_nc()
        if not nc.is_finalized():
            nc.finalize()
        _CACHED_NC = nc
    return _CACHED_NC


def _make_in_maps(x, max_, min_, ycbcr_w):
    import ml_dtypes

    bf16 = ml_dtypes.bfloat16
    x16 = np.asarray(x, np.float32).astype(bf16)
    # block-pixel layout: [B, (ci,i,j)=192, blk=(hb,wb)=4096]
    xd = x16.reshape(-1, 3, 64, 8, 64, 8)  # s, ci, hb, i, wb, j
    xd = np.ascontiguousarray(xd.transpose(0, 1, 3, 5, 2, 4))  # s, ci, i, j, hb, wb
    xd = xd.reshape(-1, 192, NBLK)
    ones = np.ones((xd.shape[0], 1, NBLK), bf16)
    xa = np.ascontiguousarray(xd[:, 0:128])
    xb = np.ascontiguousarray(np.concatenate([xd[:, 128:192], ones], axis=1))

    s_aff, b_aff = _affine_coeffs(max_, min_)  # [B, 192] f64 (co,u,v)
    D = _dct_basis()  # [u, i] f64
    y = np.asarray(ycbcr_w, np.float64)  # [co, ci]
    # K[(ci,i,j), (co,u,v)] * s_aff[smp, (co,u,v)]
    kbase = np.einsum("oc,ui,vj->cijouv", y, D, D).reshape(192, 192)
    ks = kbase[None, :, :] * s_aff[:, None, :]  # [B, 192, 192]
    k1 = ks[:, 0:128]
    k2 = np.concatenate([ks[:, 128:192], b_aff[:, None, :]], axis=1)  # [B, 65, 192]

    in_maps = []
    for core in range(NCORES):
        sl = slice(core * BPC, (core + 1) * BPC)
        in_maps.append(
            {
                "xa": xa[sl],
                "xb": xb[sl],
                "k1": k1[sl].astype(bf16),
                "k2": k2[sl].astype(bf16),
            }
        )
    return in_maps


def kernel(x, max_, min_, ycbcr_w, dct_w):
    from concourse.bass_utils import run_bass_kernel_spmd

    nc = _get_nc()
    in_maps = _make_in_maps(x, max_, min_, ycbcr_w)
    res = run_bass_kernel_spmd(nc, in_maps, core_ids=list(range(NCORES)))
    out = np.concatenate([res.results[i]["out"] for i in range(NCORES)], axis=0)
    return _untangle(out)


def _untangle(dev_out):
    """[B, p=128, g=32, 192] device layout -> [B, 192, 64, 64] f32."""
    v = np.asarray(dev_out).astype(np.float32)
    v = v.transpose(0, 2, 1, 3)  # s, g, p, (co,u,v) ; blk = g*128+p = hb*64+wb
    v = v.reshape(-1, 64, 64, 3, 8, 8)  # s, hb, wb, co, u, v
    v = v.transpose(0, 3, 4, 5, 1, 2)  # s, co, u, v, hb, wb
    return np.ascontiguousarray(v.reshape(-1, 192, 64, 64))
